# revision 29
# baseline (speedup 1.0000x reference)
"""Trainium2 Bass kernel for a 4-layer GRU stack with per-step additive
self-attention over the layer hiddens (FBRNN).

Strategy: data-parallel over batch B=64 across 8 NeuronCores (8 batch rows
per core, no cross-core communication inside the recurrence). Per core:

  - Everything lives in a [feature-on-partitions, batch-on-free] layout so
    the GRU elementwise runs on 128 DVE/ACT lanes.
  - GRU matmuls: stationary operand = bf16 weight tiles [128,128] (FWL),
    moving operand = bf16 activations [128, 8]. PSUM accumulates fp32.
  - All biases are folded away: layer-0 input bias into the prologue GEMM,
    recurrent biases are preloaded into PSUM (ACT copy) and every gate
    matmul accumulates with start=False on top.
  - gi and gh share PSUM slots for the r,z gates (single accumulation),
    removing the explicit adds.
  - State is stored as h_half = 0.5*h and the n-gate rows of W_hh are
    pre-scaled by 0.5 host-side, so the sigmoid/blend chain needs only
    scalar_tensor_tensor ops:  r*ghn = (tanh+1)*ghn', z*(h-n) =
    (tanh+1)*(0.5h - 0.5n).
  - Attention uses a uniform 4x4 (i,k) grid; ba enters as K=1 bias rows
    and the causal mask as a -40 additive PE row before exp (masked terms
    underflow to 0). h[3]==new[3] exactly, so i=3 needs no combine and the
    output DMA reads new_f32 directly.
  - sigmoid/tanh/exp all live in one ACT table set -> no table switches.
  - T-loop: tc.For_i with 16 steps unrolled per iteration.
"""

import os
import numpy as np
import ml_dtypes

import concourse.bass as bass
import concourse.mybir as mybir
import concourse.tile as tile
from concourse import bacc
from concourse.bass import ds, ts
from concourse.bass_utils import run_bass_kernel_spmd
from concourse.masks import make_identity

F32 = mybir.dt.float32
BF16 = mybir.dt.bfloat16
I32 = mybir.dt.int32
AF = mybir.ActivationFunctionType
ALU = mybir.AluOpType
AX = mybir.AxisListType

T, B = 512, 64
V, E, H, L, A = 32000, 512, 512, 4, 256
NCORES = 8
BC = B // NCORES            # 8 batch rows per core
TOK = T * BC                # 4096 tokens per core, (t, b) order
G3 = 3 * H                  # 1536 gate rows
MCH = G3 // 128             # 12 gate chunks
KCH = E // 128              # 4 contraction chunks (E == H)
ACH = A // 128              # 2 attention chunks
HT = H // 128               # 4 hidden chunks
UNROLL = 16
SLAB = 512                  # tokens per prologue gemm slab


def _bcast(ap, dim, count):
    """Insert a [step=0, count] free dim at position `dim` (0=partition)."""
    new = list(ap.ap)
    new.insert(dim, [0, count])
    return bass.AP(tensor=ap.tensor, offset=ap.offset, ap=new)


def _view(ap, dims):
    """Rebuild the free dims of `ap` as [(step, num), ...] outer->inner,
    keeping its partition dim."""
    new = [ap.ap[0]] + [[s, n] for s, n in dims]
    return bass.AP(tensor=ap.tensor, offset=ap.offset, ap=new)


def _off(ap, delta):
    """Shift an AP's element offset by `delta`."""
    return bass.AP(tensor=ap.tensor, offset=ap.offset + delta, ap=list(ap.ap))


def _build_kernel():
    nc = bacc.Bacc("TRN2", target_bir_lowering=False, debug=False)

    tokens_d = nc.dram_tensor("tokens32", [TOK // 128, 128], I32, kind="ExternalInput").ap()
    emb_d = nc.dram_tensor("embbf", [V, E], BF16, kind="ExternalInput").ap()
    wih0_d = nc.dram_tensor("wih0", [128, KCH, MCH, 128], BF16, kind="ExternalInput").ap()
    wih_d = nc.dram_tensor("wih", [L - 1, 128, KCH, MCH, 128], BF16, kind="ExternalInput").ap()
    whh_d = nc.dram_tensor("whh", [L, 128, KCH, MCH, 128], BF16, kind="ExternalInput").ap()
    wa_d = nc.dram_tensor("wa", [L, 128, KCH, ACH, 128], BF16, kind="ExternalInput").ap()
    va_d = nc.dram_tensor("vastk", [128, ACH, L], BF16, kind="ExternalInput").ap()
    ba_d = nc.dram_tensor("bab", [1, ACH, L, 128], BF16, kind="ExternalInput").ap()
    bimg_d = nc.dram_tensor("bimg", [L, 128, 16], F32, kind="ExternalInput").ap()
    pb_d = nc.dram_tensor("pb", [1, MCH, 128], BF16, kind="ExternalInput").ap()
    mask_d = nc.dram_tensor("maskneg", [1, 128], BF16, kind="ExternalInput").ap()
    out_d = nc.dram_tensor("out", [T * BC, H], F32, kind="ExternalOutput").ap()

    with tile.TileContext(nc) as tc:
        _emit(tc, nc, tokens_d, emb_d, wih0_d, wih_d, whh_d, wa_d, va_d, ba_d,
              bimg_d, pb_d, mask_d, out_d)
    nc.compile()
    return nc


def _emit(tc, nc, tokens_d, emb_d, wih0_d, wih_d, whh_d, wa_d, va_d, ba_d,
          bimg_d, pb_d, mask_d, out_d):
    from contextlib import ExitStack

    ctx = ExitStack()
    with ctx:
        wpool = ctx.enter_context(tc.tile_pool(name="weights", bufs=1))
        state = ctx.enter_context(tc.tile_pool(name="state", bufs=1))
        dram = ctx.enter_context(tc.tile_pool(name="dram", bufs=1, space="DRAM"))

        # ---- resident weights -------------------------------------------
        wih0_sb = wpool.tile([128, KCH, MCH, 128], BF16, tag="wih0")
        nc.sync.dma_start(out=wih0_sb, in_=wih0_d)
        wih_sb = []
        for l in range(L - 1):
            w = wpool.tile([128, KCH, MCH, 128], BF16, tag=f"wih{l}")
            nc.sync.dma_start(out=w, in_=wih_d[l])
            wih_sb.append(w)
        whh_sb = []
        for l in range(L):
            w = wpool.tile([128, KCH, MCH, 128], BF16, tag=f"whh{l}")
            nc.sync.dma_start(out=w, in_=whh_d[l])
            whh_sb.append(w)
        wa_sb = []
        for i in range(L):
            w = wpool.tile([128, KCH, ACH, 128], BF16, tag=f"wa{i}")
            nc.sync.dma_start(out=w, in_=wa_d[i])
            wa_sb.append(w)
        va_sb = wpool.tile([128, ACH, L], BF16, tag="va")
        nc.sync.dma_start(out=va_sb, in_=va_d)
        ba_bf = wpool.tile([1, ACH, L, 128], BF16, tag="bab")
        nc.sync.dma_start(out=ba_bf, in_=ba_d)
        bimg_sb = wpool.tile([128, L, 16], F32, tag="bimg")
        nc.sync.dma_start(out=bimg_sb, in_=bimg_d.rearrange("l p m -> p l m"))
        pb_sb = wpool.tile([1, MCH, 128], BF16, tag="pb")
        nc.sync.dma_start(out=pb_sb, in_=pb_d)
        maskneg_sb = wpool.tile([1, 128], BF16, tag="maskneg")
        nc.sync.dma_start(out=maskneg_sb, in_=mask_d)

        ident = wpool.tile([128, 128], BF16, tag="ident")
        make_identity(nc, ident)
        ones_sb = wpool.tile([1, 128], BF16, tag="ones")
        nc.vector.memset(ones_sb, 1.0)
        ones_slab = wpool.tile([1, SLAB], BF16, tag="ones_slab")
        nc.vector.memset(ones_slab, 1.0)
        ones8 = wpool.tile([1, BC], BF16, tag="ones8")
        nc.vector.memset(ones8, 1.0)

        # ---- recurrent state --------------------------------------------
        # layout: [128 part, L, HT, BC];  h_half = 0.5 * h
        h_half = state.tile([128, L, HT, BC], F32, tag="h_half")
        h_bf = state.tile([128, L, HT, BC], BF16, tag="h_bf")
        new_f32 = state.tile([128, L, HT, BC], F32, tag="new_f32")
        new_bf = state.tile([128, L, HT, BC], BF16, tag="new_bf")
        nc.vector.memset(h_half, 0.0)
        nc.vector.memset(h_bf, 0.0)
        nc.vector.memset(new_f32, 0.0)
        nc.vector.memset(new_bf, 0.0)

        # gi0[m, p, tok] fp32: precomputed x @ W_ih[0].T + bias0
        gi0_dram = dram.tile([MCH, 128, TOK], F32, tag="gi0")

        # ---- prologue: embedding gather + layer-0 input GEMM ------------
        with tc.tile_pool(name="prol", bufs=2) as prol, \
             tc.tile_pool(name="prol_ps", bufs=2, space="PSUM") as prol_ps, \
             tc.tile_pool(name="gemm_ps", bufs=2, space="PSUM") as gemm_ps, \
             tc.tile_pool(name="evac", bufs=2) as evac, \
             tc.tile_pool(name="x0t", bufs=2) as x0tp:
            for slab in range(TOK // SLAB):
                x0t = x0tp.tile([128, KCH, SLAB], BF16, tag="x0t")
                for g in range(SLAB // 128):
                    gt = slab * (SLAB // 128) + g
                    tok_sb = prol.tile([128, 1], I32, tag="tok")
                    nc.sync.dma_start(out=tok_sb, in_=tokens_d[gt, :, None])
                    x0 = prol.tile([128, E], BF16, tag="x0")
                    nc.gpsimd.indirect_dma_start(
                        out=x0, out_offset=None, in_=emb_d,
                        in_offset=bass.IndirectOffsetOnAxis(ap=tok_sb[:, 0:1], axis=0),
                    )
                    for k in range(KCH):
                        pst = prol_ps.tile([128, 128], BF16, space="PSUM", tag="pst")
                        nc.tensor.transpose(out=pst, in_=x0[:, ts(k, 128)], identity=ident)
                        nc.vector.tensor_copy(out=x0t[:, k, ts(g, 128)], in_=pst)
                for m in range(MCH):
                    ps = gemm_ps.tile([128, SLAB], F32, space="PSUM", tag="g0ps")
                    for k in range(KCH):
                        nc.tensor.matmul(
                            out=ps, lhsT=wih0_sb[:, k, m, :], rhs=x0t[:, k, :],
                            start=(k == 0), stop=False,
                        )
                    # bias row: pb[m] broadcast over the slab
                    nc.tensor.matmul(
                        out=ps, lhsT=pb_sb[0:1, m, :], rhs=ones_slab,
                        start=False, stop=True,
                    )
                    ev = evac.tile([128, SLAB], F32, tag="ev")
                    nc.scalar.activation(out=ev, in_=ps, func=AF.Copy)
                    nc.sync.dma_start(out=gi0_dram[m, :, ts(slab, SLAB)], in_=ev)

        # ---- PSUM flush -------------------------------------------------
        # The prologue's partial-bank start=True matmuls (transposes) leave
        # pending-zero flags on bytes they marked but never wrote; a later
        # start=False accumulate in the main loop would then see its bank
        # lazily zeroed mid-step. One full-bank start=True matmul per bank
        # marks AND clears the whole 2KB region atomically.
        with tc.tile_pool(name="flush_ps", bufs=1, space="PSUM") as fps:
            for i in range(8):
                ft = fps.tile([128, 512], F32, tag=f"fl{i}", name=f"fl{i}")
                nc.tensor.matmul(out=ft, lhsT=ones_sb, rhs=ones_slab,
                                 start=True, stop=True, skip_group_check=True)

        # ---- main recurrence --------------------------------------------
        loop_pools = ExitStack()
        with loop_pools:
            gip = loop_pools.enter_context(tc.tile_pool(name="gi", bufs=3))
            pgp = loop_pools.enter_context(tc.tile_pool(name="pg", bufs=2, space="PSUM"))
            ep = loop_pools.enter_context(tc.tile_pool(name="elem", bufs=3))
            up = loop_pools.enter_context(tc.tile_pool(name="ups", bufs=2, space="PSUM"))
            ap_ = loop_pools.enter_context(tc.tile_pool(name="attn", bufs=2))

            with tc.For_i(0, TOK, BC * UNROLL,
                          hint_engines=(mybir.EngineType.PE,
                                        mybir.EngineType.DVE,
                                        mybir.EngineType.Activation)) as iv:
                for u in range(UNROLL):
                    _step(tc, nc, iv, u, gip, pgp, ep, up, ap_,
                          wih_sb, whh_sb, wa_sb, va_sb, ba_bf, bimg_sb,
                          maskneg_sb, ones_sb, ones8, h_half, h_bf, new_f32,
                          new_bf, gi0_dram, out_d)


def _step(tc, nc, iv, u, gip, pgp, ep, up, ap_,
          wih_sb, whh_sb, wa_sb, va_sb, ba_bf, bimg_sb, maskneg_sb, ones_sb,
          ones8, h_half, h_bf, new_f32, new_bf, gi0_dram, out_d):
    tb0 = iv + u * BC  # token index of (t, b=0)

    # stream in the precomputed layer-0 gi for this step: [128, MCH, BC]
    gi_sb = gip.tile([128, MCH, BC], F32, tag="gi0s")
    nc.sync.dma_start(
        out=gi_sb,
        in_=gi0_dram[:, :, ds(tb0, BC)].rearrange("m p b -> p m b"),
    )

    # one PSUM bank holds all 4 layers: [128, L, 16, BC].
    # slots (l>=1): 0:8 rz (gi+gh+bias), 8:12 ghn' = 0.5*(ghn+bhn), 12:16 gin+bin
    # slots (l==0): 0:8 rz, 8:12 gin+bin (from gi0 stream), 12:16 ghn'
    # All matmuls accumulate with start=False on ACT-preloaded content
    # (start=True would lazily zero the whole 2KB bank = all 4 layers).
    pg = pgp.tile([128, L, 16, BC], F32, space="PSUM", tag="pg")

    def ghn_sl(l):
        return 12 if l == 0 else 8

    def gin_sl(l):
        return 8 if l == 0 else 12

    # PSUM preloads (GpSimd cannot write PSUM, so these live on ACT;
    # gate matmuls accumulate on top with start=False)
    nc.scalar.activation(out=pg[:, 0, 12:16, :],
                         in_=_bcast(bimg_sb[:, 0, 12:16], 2, BC), func=AF.Copy)
    nc.scalar.activation(out=pg[:, 0, 0:12, :], in_=gi_sb, func=AF.Copy)
    for l in range(1, L):
        nc.scalar.activation(out=pg[:, l, :, :],
                             in_=_bcast(bimg_sb[:, l, :], 2, BC), func=AF.Copy)

    def mm_gh(l, first_rz):
        # m 0:8 -> rz slots; m 8:12 -> ghn' slots
        # h[3] == new[3] exactly, so layer 3 reads last step's new_bf and the
        # attention pass never materializes h_bf[3].
        hsrc = new_bf if l == 3 else h_bf
        for m in range(MCH):
            sl = m if m < 8 else (ghn_sl(l) + m - 8)
            for k in range(KCH):
                stop = (k == KCH - 1) and (m >= 8 or l == 0)
                nc.tensor.matmul(
                    out=pg[:, l, sl, :],
                    lhsT=whh_sb[l][:, k, m, :],
                    rhs=hsrc[:, l, k, :],
                    start=False, stop=stop,
                    skip_group_check=True,
                )

    def mm_gi(l):  # l >= 1; input = new[l-1]
        for m in range(MCH):
            sl = m if m < 8 else (gin_sl(l) + m - 8)
            for k in range(KCH):
                nc.tensor.matmul(
                    out=pg[:, l, sl, :],
                    lhsT=wih_sb[l - 1][:, k, m, :],
                    rhs=new_bf[:, l - 1, k, :],
                    start=False, stop=(k == KCH - 1),
                    skip_group_check=True,
                )

    def elem(l):
        # t_rz = tanh(0.5 * rz_preact); r = (t+1)/2, z likewise
        t_rz = ep.tile([128, 8, BC], F32, tag="trz")
        nc.scalar.activation(out=t_rz, in_=pg[:, l, 0:8, :], func=AF.Tanh,
                             scale=0.5)
        # r*(ghn+bhn) = (t_r + 1) * ghn'
        rh = ep.tile([128, HT, BC], F32, tag="rh")
        nc.vector.scalar_tensor_tensor(
            out=rh, in0=t_rz[:, 0:4, :], scalar=1.0,
            in1=pg[:, l, ghn_sl(l):ghn_sl(l) + 4, :],
            op0=ALU.add, op1=ALU.mult)
        np_ = ep.tile([128, HT, BC], F32, tag="np")
        nc.vector.tensor_tensor(out=np_, in0=rh,
                                in1=pg[:, l, gin_sl(l):gin_sl(l) + 4, :],
                                op=ALU.add)
        n = ep.tile([128, HT, BC], F32, tag="n")
        nc.scalar.activation(out=n, in_=np_, func=AF.Tanh)
        # d2 = 0.5h - 0.5n ; zd = (t_z + 1) * d2 = z*(h-n) ; new = n + zd
        d2 = ep.tile([128, HT, BC], F32, tag="d2")
        nc.vector.scalar_tensor_tensor(
            out=d2, in0=n, scalar=-0.5, in1=h_half[:, l], op0=ALU.mult, op1=ALU.add)
        zd = ep.tile([128, HT, BC], F32, tag="zd")
        nc.vector.scalar_tensor_tensor(
            out=zd, in0=t_rz[:, 4:8, :], scalar=1.0, in1=d2, op0=ALU.add, op1=ALU.mult)
        nc.vector.tensor_tensor(out=new_bf[:, l], in0=n, in1=zd, op=ALU.add)

    # PE order: gh0, gh1, elem0, gi1, gh2, elem1, gi2, gh3, elem2, gi3, elem3
    mm_gh(0, True)
    mm_gh(1, False)
    elem(0)
    mm_gi(1)
    mm_gh(2, False)
    elem(1)
    mm_gi(2)
    mm_gh(3, False)
    elem(2)
    mm_gi(3)
    elem(3)

    # output row block: out[(t,b), :] = new[3]  (h[3] == new[3] exactly)
    out_stage = ap_.tile([128, BC, HT], F32, tag="ostage")
    nc.gpsimd.tensor_copy(out=out_stage,
                          in_=new_bf[:, 3].rearrange("p ht b -> p b ht"))
    nc.sync.dma_start(
        out=out_d[ds(tb0, BC), :].rearrange("b (ht p) -> p b ht", p=128),
        in_=out_stage,
    )

    # ---- attention combine ------------------------------------------
    # u[i,k,b] = Wa[i].T @ new[k] + ba[i] for the full 4x4 (i,k) grid.
    # ba goes in as K=1 bias rows; only the FIRST matmul in the bank uses
    # start=True (it marks the whole 2KB zero-region; later start=False
    # writes lazily zero their own bytes on first touch).
    u_ps = up.tile([128, ACH, L, L * BC], F32, space="PSUM", tag="ups")
    for i in range(L):
        for a2 in range(ACH):
            nc.tensor.matmul(
                out=u_ps[:, a2, i, :],
                lhsT=ba_bf[0:1, a2, i, :],
                rhs=ones_sb[0:1, 0:L * BC],
                start=(i == 0 and a2 == 0), stop=False,
                skip_group_check=True)
    for i in range(L):
        for a2 in range(ACH):
            for kc in range(KCH):
                nc.tensor.matmul(
                    out=u_ps[:, a2, i, :],
                    lhsT=wa_sb[i][:, kc, a2, :],
                    rhs=new_bf[:, :, kc, :],
                    start=False, stop=(kc == KCH - 1),
                    skip_group_check=True,
                )
    ut = ap_.tile([128, ACH, L, L * BC], BF16, tag="ut")
    nc.scalar.activation(out=ut, in_=u_ps, func=AF.Tanh)
    # e[i, (k,b)] = va[i] . ut[i]  + (-40 on masked-out k<i cols)
    e_ps = up.tile([1, L, L * BC], F32, space="PSUM", tag="eps")
    nc.tensor.matmul(out=e_ps.rearrange("p i kb -> p (i kb)"),
                     lhsT=ones_sb[0:1, 0:1], rhs=maskneg_sb,
                     start=True, stop=False, skip_group_check=True)
    for i in range(L):
        for a2 in range(ACH):
            nc.tensor.matmul(out=e_ps[0:1, i, :],
                             lhsT=va_sb[:, a2, i:i + 1],
                             rhs=ut[:, a2, i, :],
                             start=False, stop=(a2 == ACH - 1),
                             skip_group_check=True)
    # w = exp(e): masked cols underflow to ~0, so S = sum_k w needs no mask
    w = ap_.tile([1, L, L * BC], F32, tag="w")
    nc.scalar.activation(out=w, in_=e_ps, func=AF.Exp)
    w_flat = w.rearrange("p i kb -> p (i kb)")
    s_all = ap_.tile([1, L, BC], F32, tag="sall")
    nc.vector.tensor_reduce(
        out=s_all,
        in_=_view(w_flat, [(4 * BC, L), (1, BC), (BC, L)]),
        axis=AX.X, op=ALU.add)
    rs = ap_.tile([1, L, BC], F32, tag="rs")
    nc.vector.reciprocal(out=rs, in_=s_all)
    # alpha = w / S ; [1, i, k, b]
    alpha = ap_.tile([1, L, L * BC], F32, tag="alpha")
    nc.vector.tensor_tensor(
        out=_view(alpha.rearrange("p i kb -> p (i kb)"),
                  [(4 * BC, L), (BC, L), (1, BC)]),
        in0=_view(w_flat, [(4 * BC, L), (BC, L), (1, BC)]),
        in1=_view(rs.rearrange("p i b -> p (i b)"), [(BC, L), (0, L), (1, BC)]),
        op=ALU.mult)
    a_bf = ap_.tile([1, 128], BF16, tag="abf")
    nc.scalar.activation(out=a_bf, in_=alpha.rearrange("p i kb -> p (i kb)"),
                         func=AF.Copy)
    abc_ps = up.tile([128, 128], F32, space="PSUM", tag="abc")
    nc.tensor.matmul(out=abc_ps, lhsT=ones_sb, rhs=a_bf, start=True, stop=True,
                     skip_group_check=True)
    # h[i] = sum_k alpha[i,k] * new[k] for i<3 (h[3] == new[3] needs none).
    # Interleave reduce -> h_bf cast per i so next step's gh(i) can start as
    # early as possible.
    prod = ap_.tile([128, 3, HT, BC, L], F32, tag="prod")
    h_full = ap_.tile([128, 3, HT, BC], F32, tag="hfull")
    new_flat = new_bf.rearrange("p l ht b -> p (l ht b)")
    abc_flat = abc_ps.rearrange("p x -> p x")
    for i in range(3):
        nc.vector.tensor_tensor(
            out=prod[:, i],
            in0=_view(new_flat, [(BC, HT), (1, BC), (HT * BC, L)]),
            in1=_view(_off(abc_flat, i * L * BC),
                      [(0, HT), (1, BC), (BC, L)]),
            op=ALU.mult)
        nc.vector.tensor_reduce(out=h_full[:, i], in_=prod[:, i],
                                axis=AX.X, op=ALU.add)
        nc.scalar.activation(out=h_bf[:, i], in_=h_full[:, i], func=AF.Copy)
    # h_half for the z-blend (not urgent: consumed mid-elem next step)
    nc.scalar.activation(
        out=h_half[:, 0:3].rearrange("p l ht b -> p (l ht b)"),
        in_=h_full.rearrange("p l ht b -> p (l ht b)"),
        func=AF.Copy, scale=0.5)
    nc.scalar.activation(
        out=h_half[:, 3].rearrange("p ht b -> p (ht b)"),
        in_=new_bf[:, 3].rearrange("p ht b -> p (ht b)"),
        func=AF.Copy, scale=0.5)


_NC_CACHE = {}


def _get_nc():
    if "nc" not in _NC_CACHE:
        _NC_CACHE["nc"] = _build_kernel()
    return _NC_CACHE["nc"]


def _prep_inputs(tokens, emb, W_ih, W_hh, b_ih, b_hh, Wa, ba, va):
    """Host-side input marshalling (weight layout/dtype only, no compute)."""
    bf = ml_dtypes.bfloat16
    emb_bf = np.ascontiguousarray(np.asarray(emb, np.float32).astype(bf))

    def lhsT_layout(wT):  # [K, M] -> [128, KCH, MCH, 128]
        K, M = wT.shape
        return np.ascontiguousarray(
            wT.reshape(K // 128, 128, M // 128, 128).transpose(1, 0, 2, 3).astype(bf))

    wih_t = [lhsT_layout(np.asarray(W_ih[l], np.float32).T) for l in range(L)]
    # W_hh with the n-gate rows (1024:1536) pre-scaled by 0.5
    whh_t = []
    for l in range(L):
        w = np.asarray(W_hh[l], np.float32).copy()
        w[1024:, :] *= 0.5
        whh_t.append(lhsT_layout(w.T))
    wa_t = [lhsT_layout(np.asarray(Wa[i], np.float32)) for i in range(L)]
    va_s = np.ascontiguousarray(
        np.asarray(va, np.float32).T.reshape(ACH, 128, L).transpose(1, 0, 2).astype(bf))
    # u-matmul bias rows: ba_s[0, a2, i, p] = ba[i, a2*128 + p]
    ba_s = np.ascontiguousarray(
        np.asarray(ba, np.float32).reshape(L, ACH, 128).transpose(1, 0, 2)
        .reshape(1, ACH, L, 128).astype(bf))

    bih = np.asarray(b_ih, np.float32)
    bhh = np.asarray(b_hh, np.float32)
    bsum = bih + bhh

    # prologue bias for layer 0: rz part gets bih+bhh, n part gets bih only
    pb = np.concatenate([bsum[0, :1024], bih[0, 1024:]])
    pb_s = np.ascontiguousarray(pb.reshape(1, MCH, 128).astype(bf))

    # PSUM bias preload image [L, 128, 16]
    bimg = np.zeros((L, 128, 16), np.float32)
    for l in range(L):
        if l == 0:
            # slots 12:16 = 0.5*bhn ; 0:12 overwritten by the gi0 stream
            bimg[l, :, 12:16] = 0.5 * bhh[l, 1024:].reshape(4, 128).T
        else:
            bimg[l, :, 0:8] = bsum[l, :1024].reshape(8, 128).T
            bimg[l, :, 8:12] = 0.5 * bhh[l, 1024:].reshape(4, 128).T
            bimg[l, :, 12:16] = bih[l, 1024:].reshape(4, 128).T

    # additive mask [1, 128]: col = i*32 + k*8 + b ; -40 iff k < i
    mask = np.zeros((1, 128), np.float32)
    for i in range(L):
        for k in range(L):
            if k < i:
                mask[0, i * 32 + k * 8:i * 32 + k * 8 + 8] = -40.0
    mask = mask.astype(bf)

    return emb_bf, wih_t, whh_t, wa_t, va_s, ba_s, bimg, pb_s, mask


def kernel(tokens, emb, W_ih, W_hh, b_ih, b_hh, Wa, ba, va):
    nc = _get_nc()
    emb_bf, wih_t, whh_t, wa_t, va_s, ba_s, bimg, pb_s, mask = _prep_inputs(
        tokens, emb, W_ih, W_hh, b_ih, b_hh, Wa, ba, va)

    tok = np.asarray(tokens).astype(np.int32)  # [T, B]
    wih_arr = np.stack(wih_t[1:])
    whh_arr = np.stack(whh_t)
    wa_arr = np.stack(wa_t)

    in_maps = []
    for c in range(NCORES):
        tok_c = np.ascontiguousarray(
            tok[:, c * BC:(c + 1) * BC]).reshape(TOK // 128, 128)
        in_maps.append({
            "tokens32": tok_c,
            "embbf": emb_bf,
            "wih0": wih_t[0],
            "wih": wih_arr,
            "whh": whh_arr,
            "wa": wa_arr,
            "vastk": va_s,
            "bab": ba_s,
            "bimg": bimg,
            "pb": pb_s,
            "maskneg": mask,
        })

    trace = bool(int(os.environ.get("KERNEL_TRACE", "0")))
    res = run_bass_kernel_spmd(nc, in_maps, core_ids=list(range(NCORES)),
                               trace=trace)
    if trace:
        _NC_CACHE["last_exec_time_ns"] = res.exec_time_ns
        _NC_CACHE["last_results"] = res

    outs = []
    for c in range(NCORES):
        o = res.results[c]["out"].reshape(T, BC, H)
        outs.append(o)
    return np.concatenate(outs, axis=1)
